# revision 10
# baseline (speedup 1.0000x reference)
"""BitMGQA forward on 8 trn2 NeuronCores.

Sequence-parallel decomposition: core c owns batch b=c//4 and query rows
(c%4)*512:(c%4+1)*512. Each core computes the K/V projections for its own
512-key slice, all-gathers K^T/V inside its 4-core batch group, then runs
attention for all 16 heads over its query rows, LayerNorm, and the output
projection for those rows. Outputs are disjoint row slices -> host concat.

All projection matmuls use float32r operands (full PE rate at free-dim 512,
~1e-4 operand rounding); attention internals (q^T, k^T, v, att) are bf16.
"""

import numpy as np

import concourse.bacc as bacc
import concourse.mybir as mybir
import concourse.tile as tile
from concourse.bass_utils import run_bass_kernel_spmd

B, T, C = 2, 2048, 2048
H, KV = 16, 4
HD = C // H  # 128
KVC = HD * KV  # 512
EPS = 1e-5
R = 512  # query/key rows per core
N_CORES = 8
SCALE = 1.0 / np.sqrt(HD)

F32 = mybir.dt.float32
F32R = mybir.dt.float32r
BF16 = mybir.dt.bfloat16
AF = mybir.ActivationFunctionType
ALU = mybir.AluOpType


def build_kernel():
    nc = bacc.Bacc(
        "TRN2", target_bir_lowering=False, debug=False, num_devices=N_CORES
    )

    # Per-core inputs (host pre-transposed/tiled, see kernel() below)
    xq_d = nc.dram_tensor("xq", [128, 16, R], F32R, kind="ExternalInput").ap()
    xk_d = nc.dram_tensor("xk", [128, 16, R], F32R, kind="ExternalInput").ap()
    xv_d = nc.dram_tensor("xv", [128, 16, R], F32R, kind="ExternalInput").ap()
    # weight blocks: wq[j] = [128, 16, 128] (c-within-tile, c-tile, ch) for ch-tile j
    wq_d = nc.dram_tensor("wq", [16, 128, 16, 128], F32R, kind="ExternalInput").ap()
    wk_d = nc.dram_tensor("wk", [4, 128, 16, 128], F32R, kind="ExternalInput").ap()
    wv_d = nc.dram_tensor("wv", [128, 16, KVC], F32R, kind="ExternalInput").ap()
    wo_d = nc.dram_tensor("wo", [4, 128, 16, 512], F32R, kind="ExternalInput").ap()
    bq_d = nc.dram_tensor("bq", [128, 16], F32, kind="ExternalInput").ap()
    bk_d = nc.dram_tensor("bk", [128, 4], F32, kind="ExternalInput").ap()
    bv_d = nc.dram_tensor("bv", [1, KVC], F32R, kind="ExternalInput").ap()
    bo_d = nc.dram_tensor("bo", [1, C], F32R, kind="ExternalInput").ap()
    lnw_d = nc.dram_tensor("lnw", [128, 16], F32, kind="ExternalInput").ap()
    lnb_d = nc.dram_tensor("lnb", [128, 16], F32, kind="ExternalInput").ap()
    ones_d = nc.dram_tensor("ones", [128, 1], F32R, kind="ExternalInput").ap()
    onesb_d = nc.dram_tensor("onesb", [128, 1], BF16, kind="ExternalInput").ap()
    onesr_d = nc.dram_tensor("onesr", [1, 512], F32R, kind="ExternalInput").ap()

    out_d = nc.dram_tensor("out", [R, C], F32, kind="ExternalOutput").ap()

    with tile.TileContext(nc) as tc:
        with (
            tc.tile_pool(name="consts", bufs=1) as consts,
            tc.tile_pool(name="dram", bufs=1, space="DRAM") as dram,
            tc.tile_pool(name="big", bufs=3) as big,          # [128,16,512] f32r
            tc.tile_pool(name="wblk", bufs=2) as wblk,        # [128,16,128] f32r
            tc.tile_pool(name="qtb", bufs=1) as qtb_pool,     # [128,16,512] bf16
            tc.tile_pool(name="ktf", bufs=1) as ktf_pool,     # [128,4,2048] bf16
            tc.tile_pool(name="vfb", bufs=1) as vfb_pool,     # [128,16,512] bf16
            tc.tile_pool(name="blk", bufs=17) as blk,         # [128,512] bf16
            tc.tile_pool(name="blkf", bufs=5) as blkf,        # [128,512] f32
            tc.tile_pool(name="s1", bufs=4) as s1,            # [1,512] f32
            tc.tile_pool(name="ps", bufs=3, space="PSUM") as ps,    # [128,512]
            tc.tile_pool(name="psy", bufs=2, space="PSUM") as psy,  # [128,512]
            tc.tile_pool(name="psb", bufs=1, space="PSUM") as psb,  # [128,512]
            tc.tile_pool(name="pss", bufs=2, space="PSUM") as pss,  # [1,512]
        ):
            # ---- constants ----
            ones_col = consts.tile([128, 1], F32R)
            nc.sync.dma_start(out=ones_col[:], in_=ones_d[:])
            ones_colb = consts.tile([128, 1], BF16)
            nc.sync.dma_start(out=ones_colb[:], in_=onesb_d[:])
            ones_row = consts.tile([1, 512], F32R)
            nc.sync.dma_start(out=ones_row[:], in_=onesr_d[:])
            bq_sb = consts.tile([128, 16], F32)
            nc.sync.dma_start(out=bq_sb[:], in_=bq_d[:])
            bk_sb = consts.tile([128, 4], F32)
            nc.sync.dma_start(out=bk_sb[:], in_=bk_d[:])
            bv_sb = consts.tile([1, KVC], F32R)
            nc.sync.dma_start(out=bv_sb[:], in_=bv_d[:])
            bo_sb = consts.tile([1, C], F32R)
            nc.sync.dma_start(out=bo_sb[:], in_=bo_d[:])
            lnw_sb = consts.tile([128, 16], F32)
            nc.sync.dma_start(out=lnw_sb[:], in_=lnw_d[:])
            lnb_sb = consts.tile([128, 16], F32)
            nc.sync.dma_start(out=lnb_sb[:], in_=lnb_d[:])

            send = dram.tile([2, KVC, R], F32)
            recv = dram.tile([4, 2, KVC, R], F32)

            # ---- K projection (k^T layout: [ch, rows]) ----
            xk = big.tile([128, 16, R], F32R, tag="big")
            nc.sync.dma_start(out=xk[:], in_=xk_d[:])
            for j in range(4):
                wkb = wblk.tile([128, 16, 128], F32R, tag="wblk")
                nc.sync.dma_start(out=wkb[:], in_=wk_d[j])
                ps_k = ps.tile([128, 512], F32, tag="ps")
                for i in range(16):
                    nc.tensor.matmul(
                        ps_k[:], wkb[:, i, :], xk[:, i, :],
                        start=(i == 0), stop=(i == 15), skip_group_check=True,
                    )
                stg = blkf.tile([128, 512], F32, tag="blkf")
                nc.scalar.activation(
                    stg[:], ps_k[:], AF.Identity, bias=bk_sb[:, j:j + 1]
                )
                nc.sync.dma_start(out=send[0, j * 128:(j + 1) * 128, :], in_=stg[:])

            # ---- V projection (natural layout: [rows, ch]) ----
            xv = big.tile([128, 16, R], F32R, tag="big")
            nc.sync.dma_start(out=xv[:], in_=xv_d[:])
            wv = big.tile([128, 16, KVC], F32R, tag="big")
            nc.sync.dma_start(out=wv[:], in_=wv_d[:])
            for rt in range(4):
                ps_v = ps.tile([128, 512], F32, tag="ps")
                nc.tensor.matmul(
                    ps_v[:], ones_row[0:1, 0:128], bv_sb[0:1, :],
                    start=True, stop=False,
                )
                for i in range(16):
                    nc.tensor.matmul(
                        ps_v[:], xv[:, i, rt * 128:(rt + 1) * 128],
                        wv[:, i, :], start=False, stop=(i == 15),
                        skip_group_check=True,
                    )
                stg = blkf.tile([128, 512], F32, tag="blkf")
                nc.scalar.activation(stg[:], ps_v[:], AF.Copy)
                nc.sync.dma_start(out=send[1, rt * 128:(rt + 1) * 128, :], in_=stg[:])

            # ---- AllGather K^T/V within the 4-core batch group ----
            nc.gpsimd.collective_compute(
                "AllGather",
                ALU.bypass,
                replica_groups=[[0, 1, 2, 3], [4, 5, 6, 7]],
                ins=[send.opt()],
                outs=[recv.opt()],
            )

            # ---- Q projection (q^T layout, scale folded in by host) ----
            xq = big.tile([128, 16, R], F32R, tag="big")
            nc.sync.dma_start(out=xq[:], in_=xq_d[:])
            qtb = qtb_pool.tile([128, 16, R], BF16)
            for j in range(16):
                wqb = wblk.tile([128, 16, 128], F32R, tag="wblk")
                nc.sync.dma_start(out=wqb[:], in_=wq_d[j])
                ps_q = ps.tile([128, 512], F32, tag="ps")
                for i in range(16):
                    nc.tensor.matmul(
                        ps_q[:], wqb[:, i, :], xq[:, i, :],
                        start=(i == 0), stop=(i == 15), skip_group_check=True,
                    )
                nc.scalar.activation(
                    qtb[:, j, :], ps_q[:], AF.Identity, bias=bq_sb[:, j:j + 1]
                )

            # ---- unpack collective: k^T full (bf16) and v full (bf16) ----
            ktf = ktf_pool.tile([128, 4, T], BF16)
            for g in range(4):
                for r in range(4):
                    stg = blkf.tile([128, 512], F32, tag="blkf")
                    nc.sync.dma_start(
                        out=stg[:], in_=recv[r, 0, g * 128:(g + 1) * 128, :]
                    )
                    nc.vector.tensor_copy(ktf[:, g, r * 512:(r + 1) * 512], stg[:])
            vfb = vfb_pool.tile([128, 16, KVC], BF16)
            for r in range(4):
                for rt4 in range(4):
                    stg = blkf.tile([128, 512], F32, tag="blkf")
                    nc.sync.dma_start(
                        out=stg[:], in_=recv[r, 1, rt4 * 128:(rt4 + 1) * 128, :]
                    )
                    nc.vector.tensor_copy(vfb[:, r * 4 + rt4, :], stg[:])

            # ---- attention ----
            yt = big.tile([128, 16, R], F32R, tag="big")
            for h in range(H):
                g = h // 4
                att = []
                ps_S = pss.tile([1, 512], F32, tag="pss")
                ps_y = psy.tile([128, 512], F32, tag="psy")
                for kt in range(16):
                    ps_s = ps.tile([128, 512], F32, tag="ps")
                    nc.tensor.matmul(
                        ps_s[:], ktf[:, g, kt * 128:(kt + 1) * 128], qtb[:, h, :],
                        start=True, stop=True,
                    )
                    a = blk.tile([128, 512], BF16, tag="blk")
                    nc.scalar.activation(a[:], ps_s[:], AF.Exp)
                    att.append(a)
                for kt in range(16):
                    nc.tensor.matmul(
                        ps_S[:], ones_colb[:], att[kt][:],
                        start=(kt == 0), stop=(kt == 15), skip_group_check=True,
                    )
                for kt in range(16):
                    nc.tensor.matmul(
                        ps_y[:], vfb[:, kt, g * 128:(g + 1) * 128], att[kt][:],
                        start=(kt == 0), stop=(kt == 15), skip_group_check=True,
                    )
                rS = s1.tile([1, 512], F32R, tag="s1")
                with nc.allow_low_precision("fp32r rounding for bcast matmul"):
                    nc.vector.reciprocal(rS[:], ps_S[:])
                ps_r = psb.tile([128, 512], F32, tag="psb")
                nc.tensor.matmul(
                    ps_r[:], ones_row[0:1, 0:128], rS[:],
                    start=True, stop=True,
                )
                rSb = blkf.tile([128, 512], F32, tag="blkf")
                nc.scalar.activation(rSb[:], ps_r[:], AF.Copy)
                nc.vector.tensor_tensor(
                    yt[:, h, :], ps_y[:], rSb[:], op=ALU.mult
                )

            # ---- LayerNorm over channels (partition sums via ones matmul) ----
            ps_mu = pss.tile([1, 512], F32, tag="pss")
            ps_sq = pss.tile([1, 512], F32, tag="pss")
            for ct in range(16):
                ysq = blkf.tile([128, 512], F32R, tag="blkf")
                nc.scalar.activation(ysq[:], yt[:, ct, :], AF.Square)
                nc.tensor.matmul(
                    ps_mu[:], ones_col[:], yt[:, ct, :],
                    start=(ct == 0), stop=(ct == 15), skip_group_check=True,
                )
                nc.tensor.matmul(
                    ps_sq[:], ones_col[:], ysq[:],
                    start=(ct == 0), stop=(ct == 15), skip_group_check=True,
                )
            mu = s1.tile([1, 512], F32R, tag="s1")
            nc.vector.tensor_scalar_mul(mu[:], ps_mu[:], 1.0 / C)
            m2 = s1.tile([1, 512], F32, tag="s1")
            nc.vector.tensor_scalar_mul(m2[:], ps_sq[:], 1.0 / C)
            var = s1.tile([1, 512], F32, tag="s1")
            nc.vector.tensor_tensor(var[:], mu[:], mu[:], op=ALU.mult)
            nc.vector.tensor_tensor(var[:], m2[:], var[:], op=ALU.subtract)
            nc.vector.tensor_scalar_add(var[:], var[:], EPS)
            sd = s1.tile([1, 512], F32, tag="s1")
            nc.scalar.activation(sd[:], var[:], AF.Sqrt)
            rstd = s1.tile([1, 512], F32R, tag="s1")
            with nc.allow_low_precision("fp32r rounding for bcast matmul"):
                nc.vector.reciprocal(rstd[:], sd[:])
            # broadcast mu and rstd across partitions
            ps_r = psb.tile([128, 512], F32, tag="psb")
            nc.tensor.matmul(
                ps_r[:], ones_row[0:1, 0:128], mu[:], start=True, stop=True
            )
            mub = blkf.tile([128, 512], F32, tag="blkf")
            nc.scalar.activation(mub[:], ps_r[:], AF.Copy)
            ps_r2 = psb.tile([128, 512], F32, tag="psb")
            nc.tensor.matmul(
                ps_r2[:], ones_row[0:1, 0:128], rstd[:], start=True, stop=True
            )
            rstdb = blkf.tile([128, 512], F32, tag="blkf")
            nc.scalar.activation(rstdb[:], ps_r2[:], AF.Copy)
            for ct in range(16):
                scr = blkf.tile([128, 512], F32, tag="blkf")
                nc.vector.tensor_tensor(scr[:], yt[:, ct, :], mub[:], op=ALU.subtract)
                nc.vector.tensor_tensor(scr[:], scr[:], rstdb[:], op=ALU.mult)
                nc.vector.tensor_scalar(
                    yt[:, ct, :], scr[:],
                    lnw_sb[:, ct:ct + 1], lnb_sb[:, ct:ct + 1],
                    op0=ALU.mult, op1=ALU.add,
                )

            # ---- output projection (natural layout: [rows, ch]) ----
            for jb in range(4):
                wob = big.tile([128, 16, 512], F32R, tag="big")
                nc.sync.dma_start(out=wob[:], in_=wo_d[jb])
                for m in range(4):
                    ps_o = ps.tile([128, 512], F32, tag="ps")
                    nc.tensor.matmul(
                        ps_o[:], ones_row[0:1, 0:128],
                        bo_sb[0:1, jb * 512:(jb + 1) * 512],
                        start=True, stop=False,
                    )
                    for i in range(16):
                        nc.tensor.matmul(
                            ps_o[:], yt[:, i, m * 128:(m + 1) * 128],
                            wob[:, i, :], start=False, stop=(i == 15),
                            skip_group_check=True,
                        )
                    osb = blkf.tile([128, 512], F32, tag="blkf")
                    nc.scalar.activation(osb[:], ps_o[:], AF.Copy)
                    nc.sync.dma_start(
                        out=out_d[m * 128:(m + 1) * 128, jb * 512:(jb + 1) * 512],
                        in_=osb[:],
                    )

    nc.compile()
    return nc


_NC_CACHE = None


def _get_nc():
    global _NC_CACHE
    if _NC_CACHE is None:
        _NC_CACHE = build_kernel()
    return _NC_CACHE


def _prep_shared(Wq, bq, Wk, bk, Wv, bv, ln_w, ln_b, Wo, bo):
    s = np.float32(SCALE)
    WqT = np.ascontiguousarray(Wq.T) * s  # [c, ch], scale folded into q
    # wq[j, p, i, cc] = WqT[i*128+p, j*128+cc]
    wq = np.ascontiguousarray(WqT.reshape(16, 128, 16, 128).transpose(2, 1, 0, 3))
    WkT = np.ascontiguousarray(Wk.T)  # [2048, 512]
    wk = np.ascontiguousarray(WkT.reshape(16, 128, 4, 128).transpose(2, 1, 0, 3))
    WvT = np.ascontiguousarray(Wv.T)  # [2048, 512]
    wv = np.ascontiguousarray(WvT.reshape(16, 128, KVC).transpose(1, 0, 2))
    WoT = np.ascontiguousarray(Wo.T)  # [2048, 2048]
    wo = np.ascontiguousarray(WoT.reshape(16, 128, 4, 512).transpose(2, 1, 0, 3))
    return {
        "wq": wq,
        "wk": wk,
        "wv": wv,
        "wo": wo,
        "bq": np.ascontiguousarray((bq * s).reshape(16, 128).T),
        "bk": np.ascontiguousarray(bk.reshape(4, 128).T),
        "bv": np.ascontiguousarray(bv.reshape(1, KVC)),
        "bo": np.ascontiguousarray(bo.reshape(1, C)),
        "lnw": np.ascontiguousarray(ln_w.reshape(16, 128).T),
        "lnb": np.ascontiguousarray(ln_b.reshape(16, 128).T),
        "ones": np.ones((128, 1), np.float32),
        "onesr": np.ones((1, 512), np.float32),
    }


def _xt_tiled(x):
    # x [R, C] -> x^T tiled [128, 16, R]
    xT = np.ascontiguousarray(x.T)  # [C, R]
    return np.ascontiguousarray(xT.reshape(16, 128, R).transpose(1, 0, 2))


def kernel(
    query, key, value, Wq, bq, Wk, bk, Wv, bv, ln_w, ln_b, Wo, bo
):
    import ml_dtypes

    query = np.asarray(query, np.float32)
    key = np.asarray(key, np.float32)
    value = np.asarray(value, np.float32)

    nc = _get_nc()
    shared = _prep_shared(
        np.asarray(Wq, np.float32), np.asarray(bq, np.float32),
        np.asarray(Wk, np.float32), np.asarray(bk, np.float32),
        np.asarray(Wv, np.float32), np.asarray(bv, np.float32),
        np.asarray(ln_w, np.float32), np.asarray(ln_b, np.float32),
        np.asarray(Wo, np.float32), np.asarray(bo, np.float32),
    )
    shared["onesb"] = np.ones((128, 1), ml_dtypes.bfloat16)

    in_maps = []
    for c in range(N_CORES):
        b = c // 4
        r0 = (c % 4) * R
        m = dict(shared)
        m["xq"] = _xt_tiled(query[b, r0:r0 + R, :])
        m["xk"] = _xt_tiled(key[b, r0:r0 + R, :])
        m["xv"] = _xt_tiled(value[b, r0:r0 + R, :])
        in_maps.append(m)

    res = run_bass_kernel_spmd(nc, in_maps, core_ids=list(range(N_CORES)))

    out = np.empty((B, T, C), np.float32)
    for c in range(N_CORES):
        b = c // 4
        r0 = (c % 4) * R
        out[b, r0:r0 + R, :] = res.results[c]["out"]
    return out
